# revision 11
# baseline (speedup 1.0000x reference)
"""Trainium2 Bass kernel for nn_BlockDiagonalLinearAlignment.

Math: y = x @ A, where A is a 128x128 block-diagonal matrix assembled from
dense / diagonal / low-rank 16x16 blocks, followed by row-wise L2
normalization: out = y / (||y||_2 + 1e-8).

Strategy (pure data parallel over the batch axis, 8 cores), v3 bf16:
  - rel-err budget is 2e-2 -> compute in bf16. Host casts x to bf16 AND
    pre-transposes each core shard to feature-major xT [128, 32768] so the
    kernel needs no PE transpose and input HBM traffic halves.
  - per half-chunk (16 tiles of 128 rows): PE matmuls -> y row-major in
    PSUM f32; ACT copies y to SBUF bf16 (frees PSUM fast); DVE bn_stats
    on the bf16 copy gives per-tile segmented (even/odd) mean/var in one
    pass -> n2 = ctv_e + ctv_o + 64*(mean_e^2 + mean_o^2).
  - norm finishing per chunk: tensor ops on [128, 32] stats, ACT sqrt,
    DVE reciprocal emitting *duplicated pairs* [128, 32, 2] bf16.
  - scale: out = y * rnorm with rnorm pairs broadcast via a stride-0
    middle dim and a step-1 innermost [2] dim, keeping DVE in its 2x bf16
    mode; the scale is split between DVE (SGT tiles/half) and GPSIMD.
  - out DMA per chunk in (partition, tile, feat) layout; host reorders
    back to row-major and upcasts to f32.
  - HBM traffic 16.8MB/core vs 32MB for the f32 version.
"""

import contextlib
import functools
import sys

for _p in ("/opt/trn_rl_repo",):
    if _p not in sys.path:
        sys.path.append(_p)

import numpy as np
import ml_dtypes

import concourse.bacc as bacc
import concourse.bass as bass
import concourse.tile as tile
from concourse import bass_utils, mybir

B = 262144
D = 128
BS = 16
K = 8
N_CORES = 8
ROWS_PER_CORE = B // N_CORES  # 32768

DENSE = (0, 3, 6)
DIAG = (1, 4, 7)
LR = (2, 5)

F32 = mybir.dt.float32
BF16 = mybir.dt.bfloat16
NP_BF16 = ml_dtypes.bfloat16

P = 128
CHUNK = 4096            # rows per DMA chunk (per core)
GT = 16                 # tiles per PSUM group / half-chunk
SGT = 0                 # tiles per half scaled on DVE; GT-SGT go to GPSIMD
BUFS = dict(inpool=3, outpool=3, ypool=6, smalls=14, ps=2)
MULT = mybir.AluOpType.mult
ADD = mybir.AluOpType.add


def _assemble_A(W_dense, s_diag, U, V):
    """Full 128x128 block-diagonal transform, y = x @ A."""
    A = np.zeros((D, D), dtype=np.float32)
    for i, k in enumerate(DENSE):
        A[k * BS:(k + 1) * BS, k * BS:(k + 1) * BS] = W_dense[i].T
    for i, k in enumerate(DIAG):
        A[k * BS:(k + 1) * BS, k * BS:(k + 1) * BS] = np.diag(s_diag[i])
    for i, k in enumerate(LR):
        A[k * BS:(k + 1) * BS, k * BS:(k + 1) * BS] = V[i] @ U[i].T
    return A


def _kernel_body(ctx, tc, out_ap, xT_ap, amat_ap, rows, chunk):
    nc = tc.nc
    T = chunk // P                 # tiles per chunk
    H = T // GT                    # PSUM groups (halves) per chunk
    nchunks = rows // chunk
    assert T % GT == 0 and rows % chunk == 0

    consts = ctx.enter_context(tc.tile_pool(name="consts", bufs=1))
    amat = consts.tile([P, P], BF16)
    nc.sync.dma_start(out=amat, in_=amat_ap)

    inpool = ctx.enter_context(tc.tile_pool(name="inpool", bufs=BUFS["inpool"]))
    outpool = ctx.enter_context(tc.tile_pool(name="outpool", bufs=BUFS["outpool"]))
    ypool = ctx.enter_context(tc.tile_pool(name="ypool", bufs=BUFS["ypool"]))
    smalls = ctx.enter_context(tc.tile_pool(name="smalls", bufs=BUFS["smalls"]))
    ps = ctx.enter_context(tc.tile_pool(name="ps", bufs=BUFS["ps"], space="PSUM"))

    for c in range(nchunks):
        in_sb = inpool.tile([P, chunk], BF16)
        nc.sync.dma_start(out=in_sb, in_=xT_ap[:, c * chunk:(c + 1) * chunk])
        out_sb = outpool.tile([P, T, D], BF16)

        n2 = smalls.tile([P, H, GT], F32)
        ysbs = []
        for h in range(H):
            y_ps = ps.tile([P, GT, D], F32)
            for t in range(GT):
                g = h * GT + t
                nc.tensor.matmul(
                    y_ps[:, t], lhsT=in_sb[:, g * P:(g + 1) * P], rhs=amat,
                    start=True, stop=True,
                )
            y_sb = ypool.tile([P, GT, D], BF16)
            nc.scalar.copy(y_sb, y_ps)          # frees PSUM for next group
            sq = ypool.tile([P, GT, D], BF16)
            nc.vector.tensor_mul(sq, y_sb, y_sb)   # 2x bf16 mode
            nc.vector.tensor_reduce(
                n2[:, h], sq, axis=mybir.AxisListType.X, op=ADD,
            )
            ysbs.append(y_sb)

        nrm = smalls.tile([P, T], F32)
        nc.scalar.sqrt(nrm, n2.rearrange("p h g -> p (h g)"))
        rp = smalls.tile([P, T], F32)
        nc.vector.reciprocal(rp, nrm)

        for h in range(H):
            y_sb = ysbs[h]
            rph = rp[:, h * GT:(h + 1) * GT]    # [P, GT]
            rb = rph.unsqueeze(2).broadcast_to([P, GT, D])
            if SGT > 0:
                nc.vector.tensor_mul(
                    out_sb[:, h * GT:h * GT + SGT, :],
                    y_sb[:, 0:SGT], rb[:, 0:SGT],
                )
            if SGT < GT:
                nc.gpsimd.tensor_mul(
                    out_sb[:, h * GT + SGT:(h + 1) * GT, :],
                    y_sb[:, SGT:GT], rb[:, SGT:GT],
                )

        nc.sync.dma_start(out=out_ap[c], in_=out_sb)


@functools.lru_cache(maxsize=4)
def _build(rows, chunk):
    nc = bacc.Bacc(
        "TRN2",
        target_bir_lowering=False,
        debug=False,
        num_devices=1,
    )
    nchunks = rows // chunk
    T = chunk // P
    xT_t = nc.dram_tensor("xT", [P, rows], BF16, kind="ExternalInput").ap()
    a_t = nc.dram_tensor("amat", [D, D], BF16, kind="ExternalInput").ap()
    o_t = nc.dram_tensor("out", [nchunks, P, T * D], BF16,
                         kind="ExternalOutput").ap()
    with tile.TileContext(nc) as tc, contextlib.ExitStack() as ctx:
        _kernel_body(ctx, tc, o_t, xT_t, a_t, rows, chunk)
    nc.compile()
    return nc


def _run(x, A, trace=False, trace_cores=None):
    nc = _build(ROWS_PER_CORE, CHUNK)
    # host-side shard prep: per core, feature-major bf16 [128, ROWS_PER_CORE]
    xs = x.reshape(N_CORES, ROWS_PER_CORE, D).astype(NP_BF16)
    xTs = [np.ascontiguousarray(xs[i].T) for i in range(N_CORES)]
    A16 = A.astype(NP_BF16)
    in_maps = [{"xT": xTs[i], "amat": A16} for i in range(N_CORES)]
    res = bass_utils.run_bass_kernel_spmd(
        nc, in_maps, core_ids=list(range(N_CORES)),
        trace=trace, trace_cores=trace_cores,
    )
    nchunks = ROWS_PER_CORE // CHUNK
    T = CHUNK // P
    outs = []
    for r in res.results:
        o = np.asarray(r["out"])  # [nchunks, P, T*D] bf16
        o = o.reshape(nchunks, P, T, D).transpose(0, 2, 1, 3)
        outs.append(o.reshape(ROWS_PER_CORE, D))
    out = np.concatenate(outs, axis=0).astype(np.float32)
    return out, res


def kernel(x, W_dense, s_diag, U, V):
    A = _assemble_A(
        np.asarray(W_dense, dtype=np.float32),
        np.asarray(s_diag, dtype=np.float32),
        np.asarray(U, dtype=np.float32),
        np.asarray(V, dtype=np.float32),
    )
    out, _ = _run(np.asarray(x, dtype=np.float32), A)
    return out


# revision 12
# speedup vs baseline: 1.1267x; 1.1267x over previous
"""Trainium2 Bass kernel for nn_BlockDiagonalLinearAlignment.

Math: y = x @ A, where A is a 128x128 block-diagonal matrix assembled from
dense / diagonal / low-rank 16x16 blocks, followed by row-wise L2
normalization: out = y / (||y||_2 + 1e-8).

Strategy (pure data parallel over the batch axis, 8 cores), v3 bf16:
  - rel-err budget is 2e-2 -> compute in bf16. Host casts x to bf16 AND
    pre-transposes each core shard to feature-major xT [128, 32768] so the
    kernel needs no PE transpose and input HBM traffic halves.
  - per half-chunk (16 tiles of 128 rows): PE matmuls -> y row-major in
    PSUM f32; ACT copies y to SBUF bf16 (frees PSUM fast); DVE bn_stats
    on the bf16 copy gives per-tile segmented (even/odd) mean/var in one
    pass -> n2 = ctv_e + ctv_o + 64*(mean_e^2 + mean_o^2).
  - norm finishing per chunk: tensor ops on [128, 32] stats, ACT sqrt,
    DVE reciprocal emitting *duplicated pairs* [128, 32, 2] bf16.
  - scale: out = y * rnorm with rnorm pairs broadcast via a stride-0
    middle dim and a step-1 innermost [2] dim, keeping DVE in its 2x bf16
    mode; the scale is split between DVE (SGT tiles/half) and GPSIMD.
  - out DMA per chunk in (partition, tile, feat) layout; host reorders
    back to row-major and upcasts to f32.
  - HBM traffic 16.8MB/core vs 32MB for the f32 version.
"""

import contextlib
import functools
import sys

for _p in ("/opt/trn_rl_repo",):
    if _p not in sys.path:
        sys.path.append(_p)

import numpy as np
import ml_dtypes

import concourse.bacc as bacc
import concourse.bass as bass
import concourse.tile as tile
from concourse import bass_utils, mybir

B = 262144
D = 128
BS = 16
K = 8
N_CORES = 8
ROWS_PER_CORE = B // N_CORES  # 32768

DENSE = (0, 3, 6)
DIAG = (1, 4, 7)
LR = (2, 5)

F32 = mybir.dt.float32
BF16 = mybir.dt.bfloat16
NP_BF16 = ml_dtypes.bfloat16

P = 128
CHUNK = 4096            # rows per DMA chunk (per core)
GT = 16                 # tiles per PSUM group / half-chunk
SGT = 2                 # tiles per half scaled on DVE; GT-SGT go to GPSIMD
BUFS = dict(inpool=3, outpool=3, ypool=6, smalls=14, ps=2)
MULT = mybir.AluOpType.mult
ADD = mybir.AluOpType.add


def _assemble_A(W_dense, s_diag, U, V):
    """Full 128x128 block-diagonal transform, y = x @ A."""
    A = np.zeros((D, D), dtype=np.float32)
    for i, k in enumerate(DENSE):
        A[k * BS:(k + 1) * BS, k * BS:(k + 1) * BS] = W_dense[i].T
    for i, k in enumerate(DIAG):
        A[k * BS:(k + 1) * BS, k * BS:(k + 1) * BS] = np.diag(s_diag[i])
    for i, k in enumerate(LR):
        A[k * BS:(k + 1) * BS, k * BS:(k + 1) * BS] = V[i] @ U[i].T
    return A


def _kernel_body(ctx, tc, out_ap, xT_ap, amat_ap, rows, chunk):
    nc = tc.nc
    T = chunk // P                 # tiles per chunk
    H = T // GT                    # PSUM groups (halves) per chunk
    nchunks = rows // chunk
    assert T % GT == 0 and rows % chunk == 0

    consts = ctx.enter_context(tc.tile_pool(name="consts", bufs=1))
    amat = consts.tile([P, P], BF16)
    nc.sync.dma_start(out=amat, in_=amat_ap)

    inpool = ctx.enter_context(tc.tile_pool(name="inpool", bufs=BUFS["inpool"]))
    outpool = ctx.enter_context(tc.tile_pool(name="outpool", bufs=BUFS["outpool"]))
    ypool = ctx.enter_context(tc.tile_pool(name="ypool", bufs=BUFS["ypool"]))
    smalls = ctx.enter_context(tc.tile_pool(name="smalls", bufs=BUFS["smalls"]))
    ps = ctx.enter_context(tc.tile_pool(name="ps", bufs=BUFS["ps"], space="PSUM"))

    for c in range(nchunks):
        in_sb = inpool.tile([P, chunk], BF16)
        nc.sync.dma_start(out=in_sb, in_=xT_ap[:, c * chunk:(c + 1) * chunk])
        out_sb = outpool.tile([P, T, D], BF16)

        n2 = smalls.tile([P, H, GT], F32)
        ysbs = []
        for h in range(H):
            y_ps = ps.tile([P, GT, D], F32)
            for t in range(GT):
                g = h * GT + t
                nc.tensor.matmul(
                    y_ps[:, t], lhsT=in_sb[:, g * P:(g + 1) * P], rhs=amat,
                    start=True, stop=True,
                )
            y_sb = ypool.tile([P, GT, D], BF16)
            nc.scalar.copy(y_sb, y_ps)          # frees PSUM for next group
            sq = ypool.tile([P, GT, D], BF16)
            nc.vector.tensor_mul(sq, y_sb, y_sb)   # 2x bf16 mode
            nc.vector.tensor_reduce(
                n2[:, h], sq, axis=mybir.AxisListType.X, op=ADD,
            )
            ysbs.append(y_sb)

        nrm = smalls.tile([P, T], F32)
        nc.scalar.sqrt(nrm, n2.rearrange("p h g -> p (h g)"))
        rp = smalls.tile([P, T], F32)
        nc.vector.reciprocal(rp, nrm)

        for h in range(H):
            y_sb = ysbs[h]
            rph = rp[:, h * GT:(h + 1) * GT]    # [P, GT]
            rb = rph.unsqueeze(2).broadcast_to([P, GT, D])
            if SGT > 0:
                nc.vector.tensor_mul(
                    out_sb[:, h * GT:h * GT + SGT, :],
                    y_sb[:, 0:SGT], rb[:, 0:SGT],
                )
            if SGT < GT:
                nc.gpsimd.tensor_mul(
                    out_sb[:, h * GT + SGT:(h + 1) * GT, :],
                    y_sb[:, SGT:GT], rb[:, SGT:GT],
                )

        nc.sync.dma_start(out=out_ap[c], in_=out_sb)


@functools.lru_cache(maxsize=4)
def _build(rows, chunk):
    nc = bacc.Bacc(
        "TRN2",
        target_bir_lowering=False,
        debug=False,
        num_devices=1,
    )
    nchunks = rows // chunk
    T = chunk // P
    xT_t = nc.dram_tensor("xT", [P, rows], BF16, kind="ExternalInput").ap()
    a_t = nc.dram_tensor("amat", [D, D], BF16, kind="ExternalInput").ap()
    o_t = nc.dram_tensor("out", [nchunks, P, T * D], BF16,
                         kind="ExternalOutput").ap()
    with tile.TileContext(nc) as tc, contextlib.ExitStack() as ctx:
        _kernel_body(ctx, tc, o_t, xT_t, a_t, rows, chunk)
    nc.compile()
    return nc


def _run(x, A, trace=False, trace_cores=None):
    nc = _build(ROWS_PER_CORE, CHUNK)
    # host-side shard prep: per core, feature-major bf16 [128, ROWS_PER_CORE]
    xs = x.reshape(N_CORES, ROWS_PER_CORE, D).astype(NP_BF16)
    xTs = [np.ascontiguousarray(xs[i].T) for i in range(N_CORES)]
    A16 = A.astype(NP_BF16)
    in_maps = [{"xT": xTs[i], "amat": A16} for i in range(N_CORES)]
    res = bass_utils.run_bass_kernel_spmd(
        nc, in_maps, core_ids=list(range(N_CORES)),
        trace=trace, trace_cores=trace_cores,
    )
    nchunks = ROWS_PER_CORE // CHUNK
    T = CHUNK // P
    outs = []
    for r in res.results:
        o = np.asarray(r["out"])  # [nchunks, P, T*D] bf16
        o = o.reshape(nchunks, P, T, D).transpose(0, 2, 1, 3)
        outs.append(o.reshape(ROWS_PER_CORE, D))
    out = np.concatenate(outs, axis=0).astype(np.float32)
    return out, res


def kernel(x, W_dense, s_diag, U, V):
    A = _assemble_A(
        np.asarray(W_dense, dtype=np.float32),
        np.asarray(s_diag, dtype=np.float32),
        np.asarray(U, dtype=np.float32),
        np.asarray(V, dtype=np.float32),
    )
    out, _ = _run(np.asarray(x, dtype=np.float32), A)
    return out


# revision 13
# speedup vs baseline: 1.1314x; 1.0042x over previous
"""Trainium2 Bass kernel for nn_BlockDiagonalLinearAlignment.

Math: y = x @ A, where A is a 128x128 block-diagonal matrix assembled from
dense / diagonal / low-rank 16x16 blocks, followed by row-wise L2
normalization: out = y / (||y||_2 + 1e-8).

Strategy (pure data parallel over the batch axis, 8 cores), v3 bf16:
  - rel-err budget is 2e-2 -> compute in bf16. Host casts x to bf16 AND
    pre-transposes each core shard to feature-major xT [128, 32768] so the
    kernel needs no PE transpose and input HBM traffic halves.
  - per half-chunk (16 tiles of 128 rows): PE matmuls -> y row-major in
    PSUM f32; ACT copies y to SBUF bf16 (frees PSUM fast); DVE bn_stats
    on the bf16 copy gives per-tile segmented (even/odd) mean/var in one
    pass -> n2 = ctv_e + ctv_o + 64*(mean_e^2 + mean_o^2).
  - norm finishing per chunk: tensor ops on [128, 32] stats, ACT sqrt,
    DVE reciprocal emitting *duplicated pairs* [128, 32, 2] bf16.
  - scale: out = y * rnorm with rnorm pairs broadcast via a stride-0
    middle dim and a step-1 innermost [2] dim, keeping DVE in its 2x bf16
    mode; the scale is split between DVE (SGT tiles/half) and GPSIMD.
  - out DMA per chunk in (partition, tile, feat) layout; host reorders
    back to row-major and upcasts to f32.
  - HBM traffic 16.8MB/core vs 32MB for the f32 version.
"""

import contextlib
import functools
import sys

for _p in ("/opt/trn_rl_repo",):
    if _p not in sys.path:
        sys.path.append(_p)

import numpy as np
import ml_dtypes

import concourse.bacc as bacc
import concourse.bass as bass
import concourse.tile as tile
from concourse import bass_utils, mybir

B = 262144
D = 128
BS = 16
K = 8
N_CORES = 8
ROWS_PER_CORE = B // N_CORES  # 32768

DENSE = (0, 3, 6)
DIAG = (1, 4, 7)
LR = (2, 5)

F32 = mybir.dt.float32
BF16 = mybir.dt.bfloat16
NP_BF16 = ml_dtypes.bfloat16

P = 128
CHUNK = 4096            # rows per DMA chunk (per core)
GT = 16                 # tiles per PSUM group / half-chunk
SGT = 2                 # tiles per half scaled on DVE; GT-SGT go to GPSIMD
BUFS = dict(inpool=3, outpool=3, ypool=6, smalls=14, ps=2)
MULT = mybir.AluOpType.mult
ADD = mybir.AluOpType.add


def _assemble_A(W_dense, s_diag, U, V):
    """Full 128x128 block-diagonal transform, y = x @ A."""
    A = np.zeros((D, D), dtype=np.float32)
    for i, k in enumerate(DENSE):
        A[k * BS:(k + 1) * BS, k * BS:(k + 1) * BS] = W_dense[i].T
    for i, k in enumerate(DIAG):
        A[k * BS:(k + 1) * BS, k * BS:(k + 1) * BS] = np.diag(s_diag[i])
    for i, k in enumerate(LR):
        A[k * BS:(k + 1) * BS, k * BS:(k + 1) * BS] = V[i] @ U[i].T
    return A


def _kernel_body(ctx, tc, out_ap, xT_ap, amat_ap, rows, chunk):
    nc = tc.nc
    T = chunk // P                 # tiles per chunk
    H = T // GT                    # PSUM groups (halves) per chunk
    nchunks = rows // chunk
    assert T % GT == 0 and rows % chunk == 0

    consts = ctx.enter_context(tc.tile_pool(name="consts", bufs=1))
    amat = consts.tile([P, P], BF16)
    nc.sync.dma_start(out=amat, in_=amat_ap)

    inpool = ctx.enter_context(tc.tile_pool(name="inpool", bufs=BUFS["inpool"]))
    outpool = ctx.enter_context(tc.tile_pool(name="outpool", bufs=BUFS["outpool"]))
    ypool = ctx.enter_context(tc.tile_pool(name="ypool", bufs=BUFS["ypool"]))
    smalls = ctx.enter_context(tc.tile_pool(name="smalls", bufs=BUFS["smalls"]))
    ps = ctx.enter_context(tc.tile_pool(name="ps", bufs=BUFS["ps"], space="PSUM"))

    for c in range(nchunks):
        in_sb = inpool.tile([P, chunk], BF16)
        nc.sync.dma_start(out=in_sb, in_=xT_ap[:, c * chunk:(c + 1) * chunk])
        out_sb = outpool.tile([P, T, D], BF16)

        n2 = smalls.tile([P, H, GT], F32)
        ysbs = []
        for h in range(H):
            y_ps = ps.tile([P, GT, D], F32)
            for t in range(GT):
                g = h * GT + t
                nc.tensor.matmul(
                    y_ps[:, t], lhsT=in_sb[:, g * P:(g + 1) * P], rhs=amat,
                    start=True, stop=True,
                )
            y_sb = ypool.tile([P, GT, D], BF16)
            nc.scalar.copy(y_sb, y_ps)          # frees PSUM for next group
            sq = ypool.tile([P, GT, D], BF16)
            nc.vector.tensor_mul(sq, y_sb, y_sb)   # 2x bf16 mode
            nc.vector.tensor_reduce(
                n2[:, h], sq, axis=mybir.AxisListType.X, op=ADD,
            )
            ysbs.append(y_sb)

        nrm = smalls.tile([P, T], F32)
        nc.scalar.sqrt(nrm, n2.rearrange("p h g -> p (h g)"))
        rp = smalls.tile([P, T], BF16)
        with nc.allow_low_precision(reason="rnorm in bf16 is fine at 2e-2 tol"):
            nc.vector.reciprocal(rp, nrm)

        for h in range(H):
            y_sb = ysbs[h]
            rph = rp[:, h * GT:(h + 1) * GT]    # [P, GT]
            rb = rph.unsqueeze(2).broadcast_to([P, GT, D])
            if SGT > 0:
                nc.vector.tensor_mul(
                    out_sb[:, h * GT:h * GT + SGT, :],
                    y_sb[:, 0:SGT], rb[:, 0:SGT],
                )
            if SGT < GT:
                nc.gpsimd.tensor_mul(
                    out_sb[:, h * GT + SGT:(h + 1) * GT, :],
                    y_sb[:, SGT:GT], rb[:, SGT:GT],
                )

        nc.sync.dma_start(out=out_ap[c], in_=out_sb)


@functools.lru_cache(maxsize=4)
def _build(rows, chunk):
    nc = bacc.Bacc(
        "TRN2",
        target_bir_lowering=False,
        debug=False,
        num_devices=1,
    )
    nchunks = rows // chunk
    T = chunk // P
    xT_t = nc.dram_tensor("xT", [P, rows], BF16, kind="ExternalInput").ap()
    a_t = nc.dram_tensor("amat", [D, D], BF16, kind="ExternalInput").ap()
    o_t = nc.dram_tensor("out", [nchunks, P, T * D], BF16,
                         kind="ExternalOutput").ap()
    with tile.TileContext(nc) as tc, contextlib.ExitStack() as ctx:
        _kernel_body(ctx, tc, o_t, xT_t, a_t, rows, chunk)
    nc.compile()
    return nc


def _run(x, A, trace=False, trace_cores=None):
    nc = _build(ROWS_PER_CORE, CHUNK)
    # host-side shard prep: per core, feature-major bf16 [128, ROWS_PER_CORE]
    xs = x.reshape(N_CORES, ROWS_PER_CORE, D).astype(NP_BF16)
    xTs = [np.ascontiguousarray(xs[i].T) for i in range(N_CORES)]
    A16 = A.astype(NP_BF16)
    in_maps = [{"xT": xTs[i], "amat": A16} for i in range(N_CORES)]
    res = bass_utils.run_bass_kernel_spmd(
        nc, in_maps, core_ids=list(range(N_CORES)),
        trace=trace, trace_cores=trace_cores,
    )
    nchunks = ROWS_PER_CORE // CHUNK
    T = CHUNK // P
    outs = []
    for r in res.results:
        o = np.asarray(r["out"])  # [nchunks, P, T*D] bf16
        o = o.reshape(nchunks, P, T, D).transpose(0, 2, 1, 3)
        outs.append(o.reshape(ROWS_PER_CORE, D))
    out = np.concatenate(outs, axis=0).astype(np.float32)
    return out, res


def kernel(x, W_dense, s_diag, U, V):
    A = _assemble_A(
        np.asarray(W_dense, dtype=np.float32),
        np.asarray(s_diag, dtype=np.float32),
        np.asarray(U, dtype=np.float32),
        np.asarray(V, dtype=np.float32),
    )
    out, _ = _run(np.asarray(x, dtype=np.float32), A)
    return out
